# revision 12
# baseline (speedup 1.0000x reference)
"""CorrelationSampler Trainium2 kernel (streaming matmul formulation).

out[b, h, w, c] = bilinear sample of corr[b, :, :, c] at grid position
(h + flow_y, w + flow_x)-ish (align_corners=True, border padding).

Strategy:
  - The bilinear blend out[p, :] = sum_k w_k[p] * corr[row_k[p], :] is a
    sparse matrix product. Host packs, per 128-position tile, a dense
    [512 window-rows, 128 pos] bf16 weight matrix (4 nonzeros per
    position); the PE array does out_tile = Wmat.T @ window.
  - Positions are sorted by gather row so each tile's 4 corner rows fit a
    512-row aligned window of the source; windows advance monotonically,
    so the correlation volume streams through SBUF exactly once (dense
    direct DMA, no indirect gathers at all).
  - 8 cores = batch (4) x channel-half (2): each core streams its
    [4096 rows, 2048 ch] bf16 slice, matmuls into PSUM (f32 accumulate),
    converts to bf16 on the scalar engine, and writes [4096, 2048] bf16.
  - Host converts corr f32->bf16 / output bf16->f32 and un-permutes rows
    (bf16 end-to-end keeps rel err ~5e-3, under the 2e-2 gate).
  - The window schedule A[t] (chunk base per tile) is computed from the
    actual flow data and shared by all 8 cores (SPMD: one program).
"""

import numpy as np

B, H, W = 4, 64, 64
HW = H * W  # 4096 source rows per batch; also 4096 channels
N_CORES = 8
CH = HW // 2  # 2048 channels per core
P = 128  # partitions
N_TILES = HW // P  # 32 position tiles per core
K_CHUNKS = 4  # 128-row chunks per tile window (512-row window)
N_RING = 8  # ring buffer slots (chunks) in SBUF
PREFETCH = 2  # tiles ahead to issue chunk loads
MM_FREE = 512  # matmul moving free dim = one PSUM bank of f32


def _host_indices_weights(flow: np.ndarray):
    """float32 replica of the reference's grid math -> corner row indices
    and bilinear corner weights, shape [B, H*W] each."""
    f32 = np.float32
    y_g, x_g = np.meshgrid(
        np.arange(H, dtype=f32), np.arange(W, dtype=f32), indexing="ij"
    )
    x_norm = (f32(2.0) * x_g / f32(W - 1) - f32(1.0)).astype(f32)
    y_norm = (f32(2.0) * y_g / f32(H - 1) - f32(1.0)).astype(f32)

    fx = flow[:, 0].astype(f32)
    fy = flow[:, 1].astype(f32)
    gx = x_norm[None] + fx / f32(W) * f32(2.0)
    gy = y_norm[None] + fy / f32(H) * f32(2.0)

    ix = np.clip((gx + f32(1.0)) * f32(0.5) * f32(W - 1), f32(0.0), f32(W - 1))
    iy = np.clip((gy + f32(1.0)) * f32(0.5) * f32(H - 1), f32(0.0), f32(H - 1))

    # floor is >= 0 after the clip; clamp to W-2/H-2 so the +1 neighbor
    # always exists. At the high border this gives weight 1.0 on the last
    # row/col -- identical result to the reference's clip formulation.
    ix0 = np.minimum(np.floor(ix), f32(W - 2)).astype(np.int32)
    iy0 = np.minimum(np.floor(iy), f32(H - 2)).astype(np.int32)
    wx = (ix - ix0.astype(f32)).astype(f32)
    wy = (iy - iy0.astype(f32)).astype(f32)

    one = f32(1.0)
    w00 = ((one - wy) * (one - wx)).astype(f32)
    w01 = ((one - wy) * wx).astype(f32)
    w10 = (wy * (one - wx)).astype(f32)
    w11 = (wy * wx).astype(f32)

    row0 = iy0 * np.int32(W) + ix0  # (iy0, ix0); +1 -> (iy0, ix0+1)
    row1 = row0 + np.int32(W)  # (iy0+1, ix0); +1 -> (iy0+1, ix0+1)

    flat = lambda a: a.reshape(B, HW)
    return flat(row0), flat(row1), flat(w00), flat(w01), flat(w10), flat(w11)


def _host_schedule(row0, row1, w00, w01, w10, w11):
    """Sort each batch's positions by row0, choose the shared per-tile
    window schedule A[t], and build the dense per-tile weight matrices.

    Returns (A, perms, wmats): A [N_TILES] int chunk bases shared by all
    cores; perms[b] the position order; wmats[b] float32
    [N_TILES*P, K_CHUNKS*P] with wmats[b][t*128+r, k*128+p] = weight of
    window row 128*(A[t]+k)+r for sorted position 128*t+p.
    """
    perms = [np.argsort(row0[b], kind="stable") for b in range(B)]
    r0s = [row0[b][perms[b]] for b in range(B)]

    A = np.empty(N_TILES, dtype=np.int64)
    K = np.empty(N_TILES, dtype=np.int64)
    for t in range(N_TILES):
        lo = min(int(r0s[b][t * P]) for b in range(B))
        hi = max(int(r0s[b][(t + 1) * P - 1]) for b in range(B)) + W + 1
        A[t] = min(lo // P, HW // P - 1)
        K[t] = hi // P - A[t] + 1
        assert K[t] <= K_CHUNKS and A[t] + K[t] <= HW // P, (t, A[t], K[t])
    # feasibility: every corner row of every tile inside its window
    for b in range(B):
        for t in range(N_TILES):
            seg = r0s[b][t * P : (t + 1) * P]
            assert seg[0] >= P * A[t] and seg[-1] + W + 1 < P * (A[t] + K[t]), (
                b,
                t,
                A[t],
                seg[0],
                seg[-1],
            )

    wmats = []
    tidx = np.arange(HW) // P  # sorted rank -> tile
    pidx = np.arange(HW) % P  # sorted rank -> slot in tile
    base = (P * A)[tidx]  # window base row per sorted rank
    for b in range(B):
        q = perms[b]
        wm = np.zeros((N_TILES * P, K_CHUNKS * P), dtype=np.float32)
        for rows, wts in (
            (row0[b][q], w00[b][q]),
            (row0[b][q] + 1, w01[b][q]),
            (row1[b][q], w10[b][q]),
            (row1[b][q] + 1, w11[b][q]),
        ):
            rr = rows - base  # offset within window [0, 512)
            wm[tidx * P + rr % P, (rr // P) * P + pidx] = wts
        wmats.append(wm)
    return A, K, perms, wmats


def _build_program(A, K):
    import concourse.bacc as bacc
    import concourse.mybir as mybir
    from concourse.tile import TileContext

    bf16 = mybir.dt.bfloat16
    f32 = mybir.dt.float32

    nc = bacc.Bacc(
        "TRN2", target_bir_lowering=False, debug=False, num_devices=N_CORES
    )
    corr = nc.dram_tensor("corr", [HW, CH], bf16, kind="ExternalInput").ap()
    wm = nc.dram_tensor(
        "wm", [N_TILES * P, K_CHUNKS * P], bf16, kind="ExternalInput"
    ).ap()
    out = nc.dram_tensor("out", [HW, CH], bf16, kind="ExternalOutput").ap()

    with TileContext(nc) as tc:
        with (
            tc.tile_pool(name="ring", bufs=N_RING) as ringp,
            tc.tile_pool(name="wmp", bufs=4) as wmp,
            tc.tile_pool(name="outp", bufs=3) as outp,
            tc.tile_pool(name="psum", bufs=2, space="PSUM") as psump,
        ):
            ring = {}

            def ensure_loaded(c):
                if c not in ring:
                    rt = ringp.tile([P, CH], bf16, tag="ring")
                    nc.sync.dma_start(out=rt[:], in_=corr[P * c : P * (c + 1), :])
                    ring[c] = rt

            for t in range(N_TILES):
                tt = min(t + PREFETCH, N_TILES - 1)
                for c in range(int(A[t]), int(A[tt]) + int(K[tt])):
                    ensure_loaded(c)
                kt = int(K[t])
                # strided slice: only the used kt*P weight columns
                wmt = wmp.tile([P, kt * P], bf16, tag="wm")
                nc.sync.dma_start(
                    out=wmt[:], in_=wm[P * t : P * (t + 1), 0 : kt * P]
                )
                ot = outp.tile([P, CH], bf16, tag="out")
                # one 4-bank PSUM tile per position tile; k outer so the
                # PE streams 4 moving slices per stationary load
                ps = psump.tile([P, CH], f32, tag="ps")
                for k in range(kt):
                    for s in range(CH // MM_FREE):
                        nc.tensor.matmul(
                            ps[:, MM_FREE * s : MM_FREE * (s + 1)],
                            lhsT=wmt[:, P * k : P * (k + 1)],
                            rhs=ring[int(A[t]) + k][
                                :, MM_FREE * s : MM_FREE * (s + 1)
                            ],
                            start=(k == 0),
                            stop=(k == kt - 1),
                        )
                # f32 PSUM -> bf16 SBUF, split across the idle engines
                nc.scalar.copy(out=ot[:, 0 : CH // 2], in_=ps[:, 0 : CH // 2])
                nc.vector.tensor_copy(ot[:, CH // 2 : CH], ps[:, CH // 2 : CH])
                nc.sync.dma_start(out=out[P * t : P * (t + 1), :], in_=ot[:])
    nc.compile()
    return nc


_cached = {}


def _get_program(A, K):
    key = (tuple(int(a) for a in A), tuple(int(k) for k in K))
    if _cached.get("key") != key:
        _cached["nc"] = _build_program(A, K)
        _cached["key"] = key
    return _cached["nc"]


def _ensure_axon_hooks_importable():
    """bass_utils imports antenv.axon_hooks when tracing is requested (e.g.
    BASS_TRACE=1). Some containers ship an antenv stub without that module;
    provide a no-op registry so tracing degrades gracefully instead of
    crashing the run."""
    import sys
    import types

    try:
        import antenv.axon_hooks  # noqa: F401
    except Exception:
        m = types.ModuleType("antenv.axon_hooks")
        m._hook = None
        m.set_axon_ntff_profile_hook = lambda h: setattr(m, "_hook", h)
        m.get_axon_ntff_profile_hook = lambda: getattr(m, "_hook", None)
        sys.modules["antenv.axon_hooks"] = m


def kernel(correlation: np.ndarray, flow: np.ndarray, _trace: bool = False):
    _ensure_axon_hooks_importable()
    import ml_dtypes
    from concourse.bass_utils import run_bass_kernel_spmd

    bf16 = ml_dtypes.bfloat16
    flow = np.asarray(flow, dtype=np.float32)
    corr_bf = (
        np.ascontiguousarray(correlation, dtype=np.float32)
        .reshape(B, HW, HW)
        .astype(bf16)
    )

    row0, row1, w00, w01, w10, w11 = _host_indices_weights(flow)
    A, Kt, perms, wmats = _host_schedule(row0, row1, w00, w01, w10, w11)

    in_maps = []
    for core in range(N_CORES):
        b, half = divmod(core, 2)
        in_maps.append(
            {
                "corr": np.ascontiguousarray(
                    corr_bf[b][:, half * CH : (half + 1) * CH]
                ),
                "wm": wmats[b].astype(bf16),
            }
        )

    nc = _get_program(A, Kt)
    extra = {"trace_cores": list(range(N_CORES))} if _trace else {}
    res = run_bass_kernel_spmd(
        nc, in_maps, core_ids=list(range(N_CORES)), trace=_trace, **extra
    )

    out = np.empty((B, HW, HW), dtype=np.float32)
    for b in range(B):
        # device rows are in row0-sorted order; scatter back
        out[b, perms[b], :CH] = res.results[2 * b]["out"]
        out[b, perms[b], CH:] = res.results[2 * b + 1]["out"]
    if _trace:
        kernel.last_results = res
    return out.reshape(B, H, W, HW)


# revision 14
# speedup vs baseline: 1.2357x; 1.2357x over previous
"""CorrelationSampler Trainium2 kernel (streaming matmul formulation).

out[b, h, w, c] = bilinear sample of corr[b, :, :, c] at grid position
(h + flow_y, w + flow_x)-ish (align_corners=True, border padding).

Strategy:
  - The bilinear blend out[p, :] = sum_k w_k[p] * corr[row_k[p], :] is a
    sparse matrix product. Host packs, per 128-position tile, a dense
    [512 window-rows, 128 pos] bf16 weight matrix (4 nonzeros per
    position); the PE array does out_tile = Wmat.T @ window.
  - Positions are sorted by gather row so each tile's 4 corner rows fit a
    512-row aligned window of the source; windows advance monotonically,
    so the correlation volume streams through SBUF exactly once (dense
    direct DMA, no indirect gathers at all).
  - 8 cores = batch (4) x channel-half (2): each core streams its
    [4096 rows, 2048 ch] bf16 slice, matmuls into PSUM (f32 accumulate),
    converts to bf16 on the scalar engine, and writes [4096, 2048] bf16.
  - Host converts corr f32->bf16 / output bf16->f32 and un-permutes rows
    (bf16 end-to-end keeps rel err ~5e-3, under the 2e-2 gate).
  - The window schedule A[t] (chunk base per tile) is computed from the
    actual flow data and shared by all 8 cores (SPMD: one program).
"""

import numpy as np

B, H, W = 4, 64, 64
HW = H * W  # 4096 source rows per batch; also 4096 channels
N_CORES = 8
CH = HW // 2  # 2048 channels per core
P = 128  # partitions
N_TILES = HW // P  # 32 position tiles per core
K_CHUNKS = 4  # 128-row chunks per tile window (512-row window)
N_RING = 8  # ring buffer slots (chunks) in SBUF
PREFETCH = 2  # tiles ahead to issue chunk loads
MM_FREE = 512  # matmul moving free dim = one PSUM bank of f32


def _host_indices_weights(flow: np.ndarray):
    """float32 replica of the reference's grid math -> corner row indices
    and bilinear corner weights, shape [B, H*W] each."""
    f32 = np.float32
    y_g, x_g = np.meshgrid(
        np.arange(H, dtype=f32), np.arange(W, dtype=f32), indexing="ij"
    )
    x_norm = (f32(2.0) * x_g / f32(W - 1) - f32(1.0)).astype(f32)
    y_norm = (f32(2.0) * y_g / f32(H - 1) - f32(1.0)).astype(f32)

    fx = flow[:, 0].astype(f32)
    fy = flow[:, 1].astype(f32)
    gx = x_norm[None] + fx / f32(W) * f32(2.0)
    gy = y_norm[None] + fy / f32(H) * f32(2.0)

    ix = np.clip((gx + f32(1.0)) * f32(0.5) * f32(W - 1), f32(0.0), f32(W - 1))
    iy = np.clip((gy + f32(1.0)) * f32(0.5) * f32(H - 1), f32(0.0), f32(H - 1))

    # floor is >= 0 after the clip; clamp to W-2/H-2 so the +1 neighbor
    # always exists. At the high border this gives weight 1.0 on the last
    # row/col -- identical result to the reference's clip formulation.
    ix0 = np.minimum(np.floor(ix), f32(W - 2)).astype(np.int32)
    iy0 = np.minimum(np.floor(iy), f32(H - 2)).astype(np.int32)
    wx = (ix - ix0.astype(f32)).astype(f32)
    wy = (iy - iy0.astype(f32)).astype(f32)

    one = f32(1.0)
    w00 = ((one - wy) * (one - wx)).astype(f32)
    w01 = ((one - wy) * wx).astype(f32)
    w10 = (wy * (one - wx)).astype(f32)
    w11 = (wy * wx).astype(f32)

    row0 = iy0 * np.int32(W) + ix0  # (iy0, ix0); +1 -> (iy0, ix0+1)
    row1 = row0 + np.int32(W)  # (iy0+1, ix0); +1 -> (iy0+1, ix0+1)

    flat = lambda a: a.reshape(B, HW)
    return flat(row0), flat(row1), flat(w00), flat(w01), flat(w10), flat(w11)


def _host_schedule(row0, row1, w00, w01, w10, w11):
    """Sort each batch's positions by row0, choose the shared per-tile
    window schedule A[t], and build the dense per-tile weight matrices.

    Returns (A, perms, wmats): A [N_TILES] int chunk bases shared by all
    cores; perms[b] the position order; wmats[b] float32
    [N_TILES*P, K_CHUNKS*P] with wmats[b][t*128+r, k*128+p] = weight of
    window row 128*(A[t]+k)+r for sorted position 128*t+p.
    """
    perms = [np.argsort(row0[b], kind="stable") for b in range(B)]
    r0s = [row0[b][perms[b]] for b in range(B)]

    A = np.empty(N_TILES, dtype=np.int64)
    K = np.empty(N_TILES, dtype=np.int64)
    for t in range(N_TILES):
        lo = min(int(r0s[b][t * P]) for b in range(B))
        hi = max(int(r0s[b][(t + 1) * P - 1]) for b in range(B)) + W + 1
        A[t] = min(lo // P, HW // P - 1)
        K[t] = hi // P - A[t] + 1
        assert K[t] <= K_CHUNKS and A[t] + K[t] <= HW // P, (t, A[t], K[t])
    # feasibility: every corner row of every tile inside its window
    for b in range(B):
        for t in range(N_TILES):
            seg = r0s[b][t * P : (t + 1) * P]
            assert seg[0] >= P * A[t] and seg[-1] + W + 1 < P * (A[t] + K[t]), (
                b,
                t,
                A[t],
                seg[0],
                seg[-1],
            )

    wmats = []
    tidx = np.arange(HW) // P  # sorted rank -> tile
    pidx = np.arange(HW) % P  # sorted rank -> slot in tile
    base = (P * A)[tidx]  # window base row per sorted rank
    for b in range(B):
        q = perms[b]
        wm = np.zeros((N_TILES * P, K_CHUNKS * P), dtype=np.float32)
        for rows, wts in (
            (row0[b][q], w00[b][q]),
            (row0[b][q] + 1, w01[b][q]),
            (row1[b][q], w10[b][q]),
            (row1[b][q] + 1, w11[b][q]),
        ):
            rr = rows - base  # offset within window [0, 512)
            wm[tidx * P + rr % P, (rr // P) * P + pidx] = wts
        wmats.append(wm)
    return A, K, perms, wmats


def _build_program(A, K):
    import concourse.bacc as bacc
    import concourse.mybir as mybir
    from concourse.tile import TileContext

    bf16 = mybir.dt.bfloat16
    f32 = mybir.dt.float32

    nc = bacc.Bacc(
        "TRN2", target_bir_lowering=False, debug=False, num_devices=N_CORES
    )
    corr = nc.dram_tensor("corr", [HW, CH], bf16, kind="ExternalInput").ap()
    wm = nc.dram_tensor(
        "wm", [N_TILES * P, K_CHUNKS * P], bf16, kind="ExternalInput"
    ).ap()
    out = nc.dram_tensor("out", [HW, CH], bf16, kind="ExternalOutput").ap()

    with TileContext(nc) as tc:
        with (
            tc.tile_pool(name="ring", bufs=N_RING) as ringp,
            tc.tile_pool(name="wmp", bufs=4) as wmp,
            tc.tile_pool(name="outp", bufs=3) as outp,
            tc.tile_pool(name="psum", bufs=8, space="PSUM") as psump,
        ):
            ring = {}

            def ensure_loaded(c):
                if c not in ring:
                    rt = ringp.tile([P, CH], bf16, tag="ring")
                    nc.sync.dma_start(out=rt[:], in_=corr[P * c : P * (c + 1), :])
                    ring[c] = rt

            for t in range(N_TILES):
                tt = min(t + PREFETCH, N_TILES - 1)
                for c in range(int(A[t]), int(A[tt]) + int(K[tt])):
                    ensure_loaded(c)
                kt = int(K[t])
                # strided slice: only the used kt*P weight columns
                wmt = wmp.tile([P, kt * P], bf16, tag="wm")
                nc.sync.dma_start(
                    out=wmt[:], in_=wm[P * t : P * (t + 1), 0 : kt * P]
                )
                ot = outp.tile([P, CH], bf16, tag="out")
                for s in range(CH // MM_FREE):
                    ps = psump.tile([P, MM_FREE], f32, tag="ps")
                    for k in range(kt):
                        nc.tensor.matmul(
                            ps[:],
                            lhsT=wmt[:, P * k : P * (k + 1)],
                            rhs=ring[int(A[t]) + k][
                                :, MM_FREE * s : MM_FREE * (s + 1)
                            ],
                            start=(k == 0),
                            stop=(k == kt - 1),
                        )
                    # f32 PSUM -> bf16 SBUF on otherwise-idle engines
                    osl = ot[:, MM_FREE * s : MM_FREE * (s + 1)]
                    if s == 0:
                        nc.scalar.copy(out=osl, in_=ps[:])
                    else:
                        nc.vector.tensor_copy(osl, ps[:])
                # store on the Activation HWDGE queue: keeps the SP queue
                # free-running on loads (no head-of-line blocking behind
                # stores that wait for casts)
                nc.scalar.dma_start(out=out[P * t : P * (t + 1), :], in_=ot[:])
    nc.compile()
    return nc


_cached = {}


def _get_program(A, K):
    key = (tuple(int(a) for a in A), tuple(int(k) for k in K))
    if _cached.get("key") != key:
        _cached["nc"] = _build_program(A, K)
        _cached["key"] = key
    return _cached["nc"]


def _ensure_axon_hooks_importable():
    """bass_utils imports antenv.axon_hooks when tracing is requested (e.g.
    BASS_TRACE=1). Some containers ship an antenv stub without that module;
    provide a no-op registry so tracing degrades gracefully instead of
    crashing the run."""
    import sys
    import types

    try:
        import antenv.axon_hooks  # noqa: F401
    except Exception:
        m = types.ModuleType("antenv.axon_hooks")
        m._hook = None
        m.set_axon_ntff_profile_hook = lambda h: setattr(m, "_hook", h)
        m.get_axon_ntff_profile_hook = lambda: getattr(m, "_hook", None)
        sys.modules["antenv.axon_hooks"] = m


def kernel(correlation: np.ndarray, flow: np.ndarray, _trace: bool = False):
    _ensure_axon_hooks_importable()
    import ml_dtypes
    from concourse.bass_utils import run_bass_kernel_spmd

    bf16 = ml_dtypes.bfloat16
    flow = np.asarray(flow, dtype=np.float32)
    corr_bf = (
        np.ascontiguousarray(correlation, dtype=np.float32)
        .reshape(B, HW, HW)
        .astype(bf16)
    )

    row0, row1, w00, w01, w10, w11 = _host_indices_weights(flow)
    A, Kt, perms, wmats = _host_schedule(row0, row1, w00, w01, w10, w11)

    in_maps = []
    for core in range(N_CORES):
        b, half = divmod(core, 2)
        in_maps.append(
            {
                "corr": np.ascontiguousarray(
                    corr_bf[b][:, half * CH : (half + 1) * CH]
                ),
                "wm": wmats[b].astype(bf16),
            }
        )

    nc = _get_program(A, Kt)
    extra = {"trace_cores": list(range(N_CORES))} if _trace else {}
    res = run_bass_kernel_spmd(
        nc, in_maps, core_ids=list(range(N_CORES)), trace=_trace, **extra
    )

    out = np.empty((B, HW, HW), dtype=np.float32)
    for b in range(B):
        # device rows are in row0-sorted order; scatter back
        out[b, perms[b], :CH] = res.results[2 * b]["out"]
        out[b, perms[b], CH:] = res.results[2 * b + 1]["out"]
    if _trace:
        kernel.last_results = res
    return out.reshape(B, H, W, HW)
